# revision 11
# baseline (speedup 1.0000x reference)
"""Trainium2 Bass kernel for nn_Block_13752485281967 (dense_transformer).

Computes, distributed over 8 NeuronCores:
    q = tokens @ Wq + bq ; k = tokens @ Wk + bk ; v = tokens @ Wv + bv
    att = softmax(q.T @ k, axis=-1)              # [E, E]
    out = att @ v.T                              # [E, T]
    return out @ Wp + bp                         # [E, T]

Algebraic restructuring: q.T @ k == Wq.T @ (tokens.T @ tokens) @ Wk, so we
compute the Gram matrix G = tokens.T @ tokens once (sharded over T rows,
all-reduced in f16), then JT = G @ Wq_shard and logits = JT.T @ Wk give the
attention logits without ever materializing q or k.

Precision: single-pass f16 compute.  The softmax logits tolerate ~0.1
absolute error because the logit distribution is extremely peaked (top-2 row
gaps are almost all >> 1).  The vT all-gather — the dominant collective at
67MB/core in f16 — is shipped as int8 instead: the host pre-scales Wv by
127/VT_S so stage 2's output is already in int8 range, the PSUM eviction
adds one f16->int8 round-to-nearest copy, and the S/127 dequant constant is
folded into the softmax 1/rowsum scale.  An end-to-end numpy simulation of
this exact quantization pipeline measures rel-L2 1.43e-2 vs the f64
reference (HW measures 1.42e-2), under the 2e-2 gate.  Halving the gathered
bytes cut measured HW time from ~5.6-7.4ms to ~4.36ms: a fake-collectives
ablation runs 2.46ms, so the schedule is at the PE roofline and the
remaining gap is AR(G, 18.9MB tri) + AG(vT) wire time at the ~20GB/s
effective collective bandwidth of this part.

The G all-reduce runs as 4 descending band chunks ({7,6},{5,4},{3,2},{1,0}
stacked vertically per chunk) so the high bands land first, matching the
band-descending stage-1b transposes and the descending stage-3 m-loop that
consume them.

Sharding: T-rows of tokens for G and vT; E-rows of att (ES=512 per core) for
the logits/attention; output row-blocks are concatenated on the host.

Biases are identically zero in this problem's setup_inputs(); bp is added on
the host if nonzero, and a nonzero bq/bk/bv falls back to exact numpy.
"""

import os
import sys

import numpy as np

for _p in ("/opt/trn_rl_repo", "/root/.axon_site/_ro/trn_rl_repo"):
    if os.path.isdir(_p) and _p not in sys.path:
        sys.path.insert(0, _p)

import concourse.mybir as mybir
import concourse.tile as tile
from concourse import bacc
from concourse.bass_utils import run_bass_kernel_spmd
from concourse.masks import make_identity

T, E = 8192, 4096
NCORES = 8
TS = T // NCORES  # 1024 token rows per core
ES = E // NCORES  # 512 att rows per core
P = 128
NBANDS = 8  # G all-reduce column bands
BW = E // NBANDS  # 512 band width

F16 = mybir.dt.float16
F32 = mybir.dt.float32
I8 = mybir.dt.int8

# Gather vT across cores as int8 (host pre-scales Wv by 127/VT_S so stage 2's
# PSUM output is already in int8 range; the S/127 dequant constant is folded
# into the softmax 1/rowsum scaling).  Predicted end-to-end rel-L2 1.43e-2
# (numpy pipeline sim) vs the 2e-2 gate; halves the AllGather bytes.
INT8_VT = True
VT_S = 5.75
AX = mybir.AxisListType.X
ALU = mybir.AluOpType
EXP = mybir.ActivationFunctionType.Exp


BANDCHUNKS = [[5, 4], [3, 2], [1, 0]]  # AR chunk grouping, descending
LOCAL_BANDS = (7, 6)  # computed locally from full tokens (no all-reduce)
_BAND_CHUNK = {}
_BAND_OFF = {}
for _ci, _bs in enumerate(BANDCHUNKS):
    _off = 0
    for _b in _bs:
        _BAND_CHUNK[_b] = _ci
        _BAND_OFF[_b] = _off
        _off += 512 * (_b + 1)
_CHUNK_ROWS = [sum(512 * (b + 1) for b in bs) for bs in BANDCHUNKS]


def _build_program(single_core=False, fake_collectives=None):
    """Build the SPMD program.

    single_core=True builds a collective-free variant (collectives replaced by
    equivalent-size local DMA copies) for cost-model timeline simulation.
    fake_collectives=True keeps num_devices=8 but replaces collectives with
    local DMA copies (wrong results; used to isolate collective cost on HW).
    """
    if fake_collectives is None:
        fake_collectives = single_core
    nc = bacc.Bacc("TRN2", num_devices=1 if single_core else NCORES)

    # ------------------------------------------------------------------ I/O
    tok_h = nc.dram_tensor("tok_h", [TS, E], F16, kind="ExternalInput")
    tokf_h = nc.dram_tensor("tokf_h", [T, E], F16, kind="ExternalInput")
    tokT_h = nc.dram_tensor("tokT_h", [E, TS], F16, kind="ExternalInput")
    wq_h = nc.dram_tensor("wq_h", [E, ES], F16, kind="ExternalInput")
    wk_h = nc.dram_tensor("wk_h", [E, E], F16, kind="ExternalInput")
    wv_h = nc.dram_tensor("wv_h", [E, E], F16, kind="ExternalInput")
    wp_h = nc.dram_tensor("wp_h", [T, T], F16, kind="ExternalInput")
    out_c = nc.dram_tensor("out", [ES, T], F32, kind="ExternalOutput")

    rg = [list(range(NCORES))]
    KO_T = TS // P  # 8  k-subtiles for the T-contraction shard
    KO_E = E // P  # 32 k-subtiles for E contractions
    KO_F = T // P  # 64 k-subtiles for the final T contraction

    with tile.TileContext(nc) as tc:
        with tc.tile_pool(name="dram", bufs=1, space="DRAM") as dram, \
             tc.tile_pool(name="const", bufs=1) as constp, \
             tc.tile_pool(name="dpool", bufs=1) as dpool:
            # Symmetric G: band b holds rows 0..(4b+4)*128 of columns
            # b*512..(b+1)*512 (the upper triangle in 512-col rectangles).
            # Bands are stacked vertically into per-chunk tiles so the
            # all-reduce runs as 4 descending chunks instead of 8 bands.
            g_par_ch = [dram.tile([r, BW], F16, name=f"g_par_ch{i}")
                        for i, r in enumerate(_CHUNK_ROWS)]
            g_full_ch = [dram.tile([r, BW], F16, name=f"g_full_ch{i}",
                                   addr_space="Shared")
                         for i, r in enumerate(_CHUNK_ROWS)]

            def g_par_band(b):
                return g_par_ch[_BAND_CHUNK[b]][
                    _BAND_OFF[b]:_BAND_OFF[b] + 512 * (b + 1), :]

            # Bands 7,6 (cols 3072:4096) live here with FULL rows, computed
            # locally from tokf_h -- no all-reduce, and stage 3 reads their
            # columns without the g_lowT transposed half.
            g_loc = dram.tile([E, 1024], F16, name="g_loc")

            def g_full_band(b):
                if b in LOCAL_BANDS:
                    return g_loc[0:512 * (b + 1), (b - 6) * 512:(b - 5) * 512]
                return g_full_ch[_BAND_CHUNK[b]][
                    _BAND_OFF[b]:_BAND_OFF[b] + 512 * (b + 1), :]
            # Transposed strict-upper tiles: g_lowT[j,k] = G[j,k] for j>k
            # (only the below-diagonal tile positions are ever read).
            g_lowT = dram.tile([E, E], F16, name="g_lowT")
            VDT = I8 if INT8_VT else F16
            vt_par_h = [dram.tile([E, 512], VDT, name=f"vt_par{h}")
                        for h in range(TS // 512)]
            vt_ag_h = [dram.tile([NCORES * E, 512], VDT, name=f"vt_ag{h}",
                                 addr_space="Shared")
                       for h in range(TS // 512)]

            ident = constp.tile([P, P], F16, name="ident")
            make_identity(nc, ident)
            # per-row softmax 1/sum, persisted to the final eviction
            d_all = dpool.tile([P, ES // P], F32, name="d_all")
            # warm the Exp activation table now so stage 5 doesn't pay the
            # implicit ACT_TABLE_LOAD on the critical path
            actw = dpool.tile([P, P], F32, name="actw")
            nc.scalar.activation(actw[:], ident[:], EXP, scale=1.0)

            # tokT + first-Wv prefetch so stage 2 starts without a stall
            with tc.tile_pool(name="vtok", bufs=1) as vtokp, \
                 tc.tile_pool(name="wvp", bufs=2) as wvp:
                tT = vtokp.tile([P, KO_E, TS], F16, name="tT")
                nc.scalar.dma_start(
                    out=tT[:], in_=tokT_h.rearrange("(ko p) t -> p ko t", p=P))
                wv3 = wv_h.rearrange("(ko p) e -> p ko e", p=P)
                wvt0 = wvp.tile([P, KO_E, 512], F16, name="wvt", tag="wvt")
                nc.scalar.dma_start(out=wvt0[:], in_=wv3[:, :, 0:512])

                # ============ Stage 1: G partial + chunked f16 all-reduce ====
                # G[i1,i2] = sum_t tok[t,i1] tok[t,i2]; lhsT=rhs=tok_c (f16).
                # Bands are computed DESCENDING (7..0) and all-reduced in 4
                # descending chunks, so the high bands (needed first by the
                # descending stage-1b/3 consumers) land earliest.
                with tc.tile_pool(name="gtok", bufs=1) as gtok, \
                     tc.tile_pool(name="gstg", bufs=4) as gstg, \
                     tc.tile_pool(name="gps", bufs=4, space="PSUM") as gps:
                    th = gtok.tile([P, KO_T, E], F16, name="th")
                    th3 = tok_h.rearrange("(ko p) e -> p ko e", p=P)
                    for kk in range(KO_T):
                        nc.sync.dma_start(out=th[:, kk], in_=th3[:, kk])
                    for ci, chunk_bands in enumerate(BANDCHUNKS):
                        for n in chunk_bands:
                            gpb = g_par_band(n)
                            for m in range(4 * (n + 1)):  # row tiles: upper rect
                                ps = gps.tile([P, BW], F32, name="gps_t", tag="gps_t")
                                for k in range(KO_T):
                                    nc.tensor.matmul(
                                        ps[:], th[:, k, m * P:(m + 1) * P],
                                        th[:, k, n * BW:(n + 1) * BW],
                                        start=(k == 0), stop=(k == KO_T - 1))
                                st = gstg.tile([P, BW], F16, name="gst", tag="gst")
                                nc.vector.tensor_copy(out=st[:], in_=ps[:])
                                st_eng = nc.sync if m % 2 == 0 else nc.scalar
                                st_eng.dma_start(
                                    out=gpb[m * P:(m + 1) * P, :], in_=st[:])
                        if fake_collectives:
                            nc.gpsimd.dma_start(out=g_full_ch[ci][:],
                                                in_=g_par_ch[ci][:])
                        else:
                            nc.gpsimd.collective_compute(
                                "AllReduce", ALU.add, replica_groups=rg,
                                ins=[g_par_ch[ci].opt()], outs=[g_full_ch[ci].opt()])

                # ============ Stage 2: vT = Wv.T @ tokens.T, split AG =======
                # (runs on PE while the G all-reduce is in flight); T-halves
                # are all-gathered separately so stage 6 can start on the
                # first half before the second arrives.
                with tc.tile_pool(name="vstg", bufs=4) as vstg, \
                     tc.tile_pool(name="vps", bufs=4, space="PSUM") as vps:
                    for nn in range(TS // 512):  # 2 T-halves
                        for mg in range(E // 512):  # 8 groups of 4 m-tiles
                            if nn == 0 and mg == 0:
                                wvt = wvt0
                            else:
                                wvt = wvp.tile([P, KO_E, 512], F16, name="wvt",
                                               tag="wvt")
                                dma_eng = nc.sync if mg % 2 == 0 else nc.scalar
                                dma_eng.dma_start(
                                    out=wvt[:],
                                    in_=wv3[:, :, mg * 512:(mg + 1) * 512])
                            for ms in range(4):
                                m = mg * 4 + ms
                                ps = vps.tile([P, 512], F32, name="vps_t", tag="vps_t")
                                for k in range(KO_E):
                                    nc.tensor.matmul(
                                        ps[:], wvt[:, k, ms * P:(ms + 1) * P],
                                        tT[:, k, nn * 512:(nn + 1) * 512],
                                        start=(k == 0), stop=(k == KO_E - 1))
                                st = vstg.tile([P, 512], F16, name="vst", tag="vst")
                                nc.vector.tensor_copy(out=st[:], in_=ps[:])
                                if INT8_VT:
                                    stq = vstg.tile([P, 512], I8, name="vsq",
                                                    tag="vsq")
                                    nc.vector.tensor_copy(out=stq[:], in_=st[:])
                                    st = stq
                                st_eng = nc.scalar if mg % 2 == 0 else nc.sync
                                st_eng.dma_start(
                                    out=vt_par_h[nn][m * P:(m + 1) * P, :],
                                    in_=st[:])
                        if fake_collectives:
                            nc.gpsimd.dma_start(
                                out=vt_ag_h[nn][0:E, :], in_=vt_par_h[nn][:])
                        else:
                            nc.gpsimd.collective_compute(
                                "AllGather", ALU.bypass, replica_groups=rg,
                                ins=[vt_par_h[nn].opt()], outs=[vt_ag_h[nn].opt()])

            # ===== Stage 1c: bands 7,6 of G from full tokens (local) ====
            # G[:, 3072:4096] = tokf.T @ tokf[:, 3072:4096], full-T
            # contraction.  f16 moving-operand max is 512 wide, so each
            # (k, row-tile) step issues two N=512 matmuls into paired PSUM
            # tiles (ISA check s3d3_mm_num_elements rejects N=1024 f16).
            tok4 = tokf_h.rearrange("(ko p) e -> p ko e", p=P)
            with tc.tile_pool(name="lch", bufs=2) as lch, \
                 tc.tile_pool(name="lst", bufs=4) as lstp, \
                 tc.tile_pool(name="lps", bufs=2, space="PSUM") as lps:
                for mp in range(E // 256):  # 16 row-pairs
                    pss = [[lps.tile([P, 512], F32, name=f"lps_t{mm}_{j}",
                                     tag=f"lps_t{mm}_{j}")
                            for j in range(2)] for mm in range(2)]
                    for kg in range(4):  # 16 k-tiles per chunk
                        ld = lch.tile([P, 16, 1280], F16, name="ld", tag="ld")
                        dma_eng = nc.sync if kg % 2 == 0 else nc.scalar
                        dma_eng.dma_start(
                            out=ld[:, :, 0:256],
                            in_=tok4[:, kg * 16:(kg + 1) * 16,
                                     mp * 256:(mp + 1) * 256])
                        dma_eng.dma_start(
                            out=ld[:, :, 256:1280],
                            in_=tok4[:, kg * 16:(kg + 1) * 16, 3072:4096])
                        for k in range(16):
                            for mm in range(2):
                                for j in range(2):
                                    nc.tensor.matmul(
                                        pss[mm][j][:],
                                        ld[:, k, mm * P:(mm + 1) * P],
                                        ld[:, k, 256 + j * 512:256 + (j + 1) * 512],
                                        start=(kg == 0 and k == 0),
                                        stop=(kg == 3 and k == 15))
                    for mm in range(2):
                        m = mp * 2 + mm
                        st = lstp.tile([P, 1024], F16, name="lgst", tag="lgst")
                        nc.vector.tensor_copy(out=st[:, 0:512], in_=pss[mm][0][:])
                        nc.vector.tensor_copy(out=st[:, 512:1024], in_=pss[mm][1][:])
                        st_eng = nc.sync if mm == 0 else nc.scalar
                        st_eng.dma_start(
                            out=g_loc[m * P:(m + 1) * P, :], in_=st[:])

            # ============ Stage 1b: transpose strict-upper G tiles ======
            # g_lowT[j,k] = g_full[k-band][k-rows, j-col].T for j > k, so
            # stage 3 can read any G column from (g_full direct) +
            # (g_lowT below-diagonal) without recomputing the lower half.
            with tc.tile_pool(name="trl", bufs=2) as trl, \
                 tc.tile_pool(name="trs", bufs=4) as trs, \
                 tc.tile_pool(name="trp", bufs=8, space="PSUM") as trp:
                for n in range(NBANDS - 1, 0, -1):  # band 0: nothing above
                    g3 = g_full_band(n).rearrange("(ko p) c -> p ko c", p=P)
                    gtr = trl.tile([P, 28, BW], F16, name="gtr", tag="gtr")
                    dma_eng = nc.sync if n % 2 == 0 else nc.scalar
                    dma_eng.dma_start(out=gtr[:, 0:4 * n], in_=g3[:, 0:4 * n, :])
                    for jj in range(4):  # col tile within band
                        j = 4 * n + jj
                        stp = trs.tile([P, 28, P], F16, name="tst", tag="tst")
                        for m in range(4 * n):  # row tiles above diagonal
                            pst = trp.tile([P, P], F16, name="tpt", tag="tpt")
                            nc.tensor.transpose(
                                pst[:], gtr[:, m, jj * P:(jj + 1) * P], ident[:])
                            nc.vector.tensor_copy(out=stp[:, m], in_=pst[:])
                        st_eng = nc.scalar if jj % 2 == 0 else nc.sync
                        st_eng.dma_start(
                            out=g_lowT[j * P:(j + 1) * P, 0:4 * n * P],
                            in_=stp[:, 0:4 * n])

            # ================= Stage 3: JT = G @ Wq_c  (f16 G, 1 pass) =====
            # Pool lifetimes: at spans stages 5-7, lg_sb 4-5, jt 3-4, oT 6-7.
            with tc.tile_pool(name="atp", bufs=1) as atp:
                at = atp.tile([P, KO_E, ES], F16, name="at")
                with tc.tile_pool(name="lgp", bufs=1) as lgp:
                    lg_sb = lgp.tile([P, ES // P, E], F32, name="lg_sb")
                    with tc.tile_pool(name="jtp", bufs=1) as jtp:
                        jt = jtp.tile([P, KO_E, ES], F16, name="jt")
                        with tc.tile_pool(name="wqp", bufs=1) as wqp, \
                             tc.tile_pool(name="gld", bufs=4) as gld, \
                             tc.tile_pool(name="jps", bufs=4, space="PSUM") as jps:
                            wqs = wqp.tile([P, KO_E, ES], F16, name="wqs")
                            nc.sync.dma_start(
                                out=wqs[:],
                                in_=wq_h.rearrange("(ko p) e -> p ko e", p=P))
                            for m in reversed(range(E // P)):  # 32 tiles, descending
                                b = m // (E // P // NBANDS)
                                mib = m % (E // P // NBANDS)
                                nup = 4 * (b + 1)  # direct rows from band b
                                gt = gld.tile([P, KO_E, P], F16, name="gt", tag="gt")
                                dma_eng = nc.sync if m % 2 == 0 else nc.scalar
                                if b in (6, 7):
                                    g3f = g_loc[:, (b - 6) * 512 + mib * P:
                                                (b - 6) * 512 + (mib + 1) * P] \
                                        .rearrange("(ko p) c -> p ko c", p=P)
                                    dma_eng.dma_start(out=gt[:], in_=g3f[:])
                                    nup = KO_E
                                else:
                                    g3 = g_full_band(b).rearrange("(ko p) c -> p ko c", p=P)
                                    dma_eng.dma_start(
                                        out=gt[:, 0:nup],
                                        in_=g3[:, :, mib * P:(mib + 1) * P])
                                if nup < KO_E:
                                    gl3 = g_lowT[nup * P:E, m * P:(m + 1) * P] \
                                        .rearrange("(ko p) c -> p ko c", p=P)
                                    dma_eng.dma_start(out=gt[:, nup:KO_E], in_=gl3[:])
                                ps = jps.tile([P, ES], F32, name="jps_t", tag="jps_t")
                                for k in range(KO_E):
                                    nc.tensor.matmul(
                                        ps[:], gt[:, k], wqs[:, k],
                                        start=(k == 0), stop=(k == KO_E - 1))
                                nc.vector.tensor_copy(out=jt[:, m], in_=ps[:])

                        # ===== Stage 4: logits = JT.T @ Wk -> SBUF f32 =====
                        with tc.tile_pool(name="wkp", bufs=4) as wkp, \
                             tc.tile_pool(name="lps", bufs=8, space="PSUM") as lps:
                            wk3 = wk_h.rearrange("(ko p) e -> p ko e", p=P)
                            for n in range(E // 512):  # 8
                                pss = [lps.tile([P, 512], F32, name=f"lps_t{m}",
                                                tag="lps_t")
                                       for m in range(ES // P)]
                                for kh in range(2):
                                    wkt = wkp.tile([P, 16, 512], F16, name="wkt",
                                                   tag="wkt")
                                    dma_eng = nc.sync if (2 * n + kh) % 2 == 0 else nc.scalar
                                    dma_eng.dma_start(
                                        out=wkt[:],
                                        in_=wk3[:, kh * 16:(kh + 1) * 16,
                                                n * 512:(n + 1) * 512])
                                    for m in range(ES // P):  # 4
                                        for k in range(16):
                                            kk = kh * 16 + k
                                            nc.tensor.matmul(
                                                pss[m][:],
                                                jt[:, kk, m * P:(m + 1) * P],
                                                wkt[:, k],
                                                start=(kh == 0 and k == 0),
                                                stop=(kh == 1 and k == 15))
                                for m in range(ES // P):
                                    nc.vector.tensor_copy(
                                        out=lg_sb[:, m, n * 512:(n + 1) * 512],
                                        in_=pss[m][:])

                    # ===== Stage 5: softmax + PE transpose of att ==========
                    # att rows stay unnormalized (exp only, f16); 1/rowsum is
                    # folded into the final-stage eviction via d_all.
                    with tc.tile_pool(name="smx", bufs=2) as smx, \
                         tc.tile_pool(name="tps", bufs=4, space="PSUM") as tps:
                        for m in range(ES // P):  # 4
                            negm = smx.tile([P, 1], F32, name="negm", tag="negm")
                            nc.vector.tensor_reduce(
                                out=negm[:], in_=lg_sb[:, m], axis=AX, op=ALU.max,
                                negate=True)
                            pexp = smx.tile([P, E], F16, name="pexp", tag="pexp")
                            ssum = smx.tile([P, 1], F32, name="ssum", tag="ssum")
                            nc.scalar.activation(
                                pexp[:], lg_sb[:, m], EXP, bias=negm[:], scale=1.0,
                                accum_out=ssum[:])
                            nc.vector.reciprocal(d_all[:, m:m + 1], ssum[:])
                            if INT8_VT:
                                nc.vector.tensor_scalar_mul(
                                    d_all[:, m:m + 1], d_all[:, m:m + 1],
                                    VT_S / 127.0)
                            for j in range(KO_E):  # 32 PE transposes [128,128]
                                pst = tps.tile([P, P], F16, name="pst", tag="pst")
                                nc.tensor.transpose(
                                    pst[:], pexp[:, j * P:(j + 1) * P], ident[:])
                                nc.vector.tensor_copy(
                                    out=at[:, j, m * P:(m + 1) * P], in_=pst[:])

                # ========= Stage 6: oT = vT(gathered) x attT ===============
                # oT[t, e1] = sum_j vT[j, t] * attT[j, e1] (unnormalized).
                with tc.tile_pool(name="oTp", bufs=1) as oTp:
                    oT = oTp.tile([P, KO_F, ES], F16, name="oT")
                    with tc.tile_pool(name="vtp", bufs=2) as vtp, \
                         tc.tile_pool(name="ops", bufs=4, space="PSUM") as ops:
                        for i6, (h, c) in enumerate(
                                [(h, c) for h in range(TS // 512)
                                 for c in range(NCORES)]):
                            vt3c = vt_ag_h[h][c * E:(c + 1) * E, :].rearrange(
                                "(ko p) t -> p ko t", p=P)
                            vtt = vtp.tile([P, KO_E, 512], VDT, name="vtt", tag="vtt")
                            dma_eng = nc.sync if i6 % 2 == 0 else nc.scalar
                            dma_eng.dma_start(out=vtt[:], in_=vt3c[:])
                            if INT8_VT:
                                vtf = vtp.tile([P, KO_E, 512], F16, name="vtf",
                                               tag="vtf")
                                nc.vector.tensor_copy(out=vtf[:], in_=vtt[:])
                                vtt = vtf
                            for ms in range(4):
                                m = (c * TS + h * 512) // P + ms
                                ps = ops.tile([P, ES], F32, name="ops_t", tag="ops_t")
                                for k in range(KO_E):
                                    nc.tensor.matmul(
                                        ps[:], vtt[:, k, ms * P:(ms + 1) * P],
                                        at[:, k],
                                        start=(k == 0), stop=(k == KO_E - 1))
                                nc.vector.tensor_copy(out=oT[:, m], in_=ps[:])

                    # ===== Stage 7: final = oT.T @ Wp (row-scaled) =========
                    with tc.tile_pool(name="wpp", bufs=3) as wpp, \
                         tc.tile_pool(name="fstg", bufs=4) as fstg, \
                         tc.tile_pool(name="fps", bufs=8, space="PSUM") as fps:
                        wp3 = wp_h.rearrange("(ko p) t -> p ko t", p=P)
                        for n in range(T // 512):  # 16
                            pss = [fps.tile([P, 512], F32, name=f"fps_t{m}",
                                            tag="fps_t")
                                   for m in range(ES // P)]
                            for kh in range(2):
                                wpt = wpp.tile([P, 32, 512], F16, name="wpt",
                                               tag="wpt")
                                dma_eng = nc.sync if (2 * n + kh) % 2 == 0 else nc.scalar
                                dma_eng.dma_start(
                                    out=wpt[:],
                                    in_=wp3[:, kh * 32:(kh + 1) * 32,
                                            n * 512:(n + 1) * 512])
                                for m in range(ES // P):  # 4
                                    for k in range(32):
                                        kk = kh * 32 + k
                                        nc.tensor.matmul(
                                            pss[m][:],
                                            oT[:, kk, m * P:(m + 1) * P],
                                            wpt[:, k],
                                            start=(kh == 0 and k == 0),
                                            stop=(kh == 1 and k == 31))
                            for m in range(ES // P):
                                st = fstg.tile([P, 512], F32, name="fst", tag="fst")
                                nc.vector.tensor_scalar_mul(
                                    st[:], pss[m][:], d_all[:, m:m + 1])
                                st_eng = nc.scalar if (n + m) % 2 == 0 else nc.sync
                                st_eng.dma_start(
                                    out=out_c[m * P:(m + 1) * P,
                                              n * 512:(n + 1) * 512],
                                    in_=st[:])

    nc.compile()
    return nc


_PROG = None
_LAST_RESULTS = None


def _get_program():
    global _PROG
    if _PROG is None:
        _PROG = _build_program()
    return _PROG


def _numpy_fallback(tokens, Wq, bq, Wk, bk, Wv, bv, Wp, bp):
    t64 = tokens.astype(np.float64)
    q = t64 @ Wq.astype(np.float64) + bq.astype(np.float64)
    k = t64 @ Wk.astype(np.float64) + bk.astype(np.float64)
    v = t64 @ Wv.astype(np.float64) + bv.astype(np.float64)
    z = q.T @ k
    z -= z.max(-1, keepdims=True)
    a = np.exp(z)
    a /= a.sum(-1, keepdims=True)
    out = a @ v.T
    return (out @ Wp.astype(np.float64) + bp.astype(np.float64)).astype(np.float32)


def _make_in_maps(tokens, Wq, Wk, Wv, Wp):
    f16 = np.float16
    wk_hi = Wk.astype(f16)
    if INT8_VT:
        Wv = Wv * np.float32(127.0 / VT_S)
    wv_hi = Wv.astype(f16)
    wp_hi = Wp.astype(f16)
    tokf16 = tokens.astype(f16)
    in_maps = []
    for c in range(NCORES):
        tok_c = tokens[c * TS:(c + 1) * TS]
        in_maps.append({
            "tok_h": tok_c.astype(f16),
            "tokf_h": tokf16,
            "tokT_h": np.ascontiguousarray(tok_c.T).astype(f16),
            "wq_h": np.ascontiguousarray(Wq[:, c * ES:(c + 1) * ES]).astype(f16),
            "wk_h": wk_hi,
            "wv_h": wv_hi,
            "wp_h": wp_hi,
        })
    return in_maps


def kernel(tokens, Wq, bq, Wk, bk, Wv, bv, Wp, bp):
    tokens = np.ascontiguousarray(np.asarray(tokens, dtype=np.float32))
    Wq = np.asarray(Wq, dtype=np.float32)
    Wk = np.asarray(Wk, dtype=np.float32)
    Wv = np.asarray(Wv, dtype=np.float32)
    Wp = np.asarray(Wp, dtype=np.float32)
    bq = np.asarray(bq, dtype=np.float32)
    bk = np.asarray(bk, dtype=np.float32)
    bv = np.asarray(bv, dtype=np.float32)
    bp = np.asarray(bp, dtype=np.float32)

    if any(np.any(b) for b in (bq, bk, bv)):
        # Never hit for this problem (biases are zeros); exact fallback.
        return _numpy_fallback(tokens, Wq, bq, Wk, bk, Wv, bv, Wp, bp)

    nc = _get_program()
    res = run_bass_kernel_spmd(nc, _make_in_maps(tokens, Wq, Wk, Wv, Wp),
                               list(range(NCORES)))
    global _LAST_RESULTS
    _LAST_RESULTS = res

    out = np.concatenate([res.results[c]["out"] for c in range(NCORES)], axis=0)
    if np.any(bp):
        out = out + bp[None, :]
    return out.astype(np.float32)


# --------------------------------------------------------------------------
# Benchmarking helpers (not used by the grading path; test.py uses these to
# measure device execution time with device-resident inputs, subtracting the
# large fixed axon/PJRT dispatch overhead via a chain-length slope).
# --------------------------------------------------------------------------


def make_exec_and_inputs(inputs):
    import jax
    import jax.core
    from jax.sharding import Mesh, NamedSharding, PartitionSpec
    from jax.experimental.shard_map import shard_map

    from concourse.bass2jax import (
        _bass_exec_p,
        install_neuronx_cc_hook,
        partition_id_tensor,
    )

    nc = _get_program()
    install_neuronx_cc_hook()
    partition_name = nc.partition_id_tensor.name if nc.partition_id_tensor else None
    in_names, out_names, out_avals, zero_outs = [], [], [], []
    for alloc in nc.m.functions[0].allocations:
        if not isinstance(alloc, mybir.MemoryLocationSet):
            continue
        name = alloc.memorylocations[0].name
        if alloc.kind == "ExternalInput":
            if name != partition_name:
                in_names.append(name)
        elif alloc.kind == "ExternalOutput":
            out_names.append(name)
            out_avals.append(
                jax.core.ShapedArray(tuple(alloc.tensor_shape), mybir.dt.np(alloc.dtype)))
            zero_outs.append(
                np.zeros(tuple(alloc.tensor_shape), mybir.dt.np(alloc.dtype)))
    n_params, n_outs = len(in_names), len(out_avals)
    all_in = in_names + out_names + ([partition_name] if partition_name else [])
    donate = tuple(range(n_params, n_params + n_outs))

    def _body(*args):
        operands = list(args)
        if partition_name:
            operands.append(partition_id_tensor())
        return tuple(_bass_exec_p.bind(
            *operands, out_avals=tuple(out_avals), in_names=tuple(all_in),
            out_names=tuple(out_names), lowering_input_output_aliases=(),
            sim_require_finite=True, sim_require_nnan=True, nc=nc))

    mesh = Mesh(np.asarray(jax.devices()[:NCORES]), ("core",))
    sharded = jax.jit(
        shard_map(_body, mesh=mesh,
                  in_specs=(PartitionSpec("core"),) * (n_params + n_outs),
                  out_specs=(PartitionSpec("core"),) * n_outs, check_rep=False),
        donate_argnums=donate, keep_unused=True)

    tokens = np.ascontiguousarray(np.asarray(inputs["tokens"], dtype=np.float32))
    in_maps = _make_in_maps(
        tokens,
        np.asarray(inputs["Wq"], dtype=np.float32),
        np.asarray(inputs["Wk"], dtype=np.float32),
        np.asarray(inputs["Wv"], dtype=np.float32),
        np.asarray(inputs["Wp"], dtype=np.float32))

    sh = NamedSharding(mesh, PartitionSpec("core"))
    concat_in = [
        np.concatenate([np.asarray(in_maps[c][nm]) for c in range(NCORES)], axis=0)
        for nm in in_names
    ]
    dev_in = [jax.device_put(a, sh) for a in concat_in]
    jax.block_until_ready(dev_in)

    import jax.numpy as jnp

    zero_makers = [
        jax.jit(
            (lambda shape=(NCORES * z.shape[0], *z.shape[1:]), dt=z.dtype:
             jnp.zeros(shape, dt)),
            out_shardings=sh)
        for z in zero_outs
    ]

    def fresh_zeros():
        zs = [mk() for mk in zero_makers]
        jax.block_until_ready(zs)
        return zs

    return sharded, dev_in, fresh_zeros


def measure_exec_time_ns(inputs, k1=4, k2=20, reps=4):
    """Device exec time per NEFF run, via chain-length slope (removes the
    fixed axon dispatch overhead). Returns (ns, last_out_arrays)."""
    import time as _time

    import jax

    sharded, dev_in, fresh_zeros = make_exec_and_inputs(inputs)
    # warmup: compile + load
    outs = sharded(*dev_in, *fresh_zeros())
    jax.block_until_ready(outs)

    def chain(k):
        zsets = [fresh_zeros() for _ in range(k)]
        t0 = _time.perf_counter()
        outs = [sharded(*dev_in, *zsets[i]) for i in range(k)]
        jax.block_until_ready(outs)
        return _time.perf_counter() - t0, outs[-1]

    t1s, t2s, last = [], [], None
    for _ in range(reps):
        t1, _o = chain(k1)
        t2, last = chain(k2)
        t1s.append(t1)
        t2s.append(t2)
    slope = (min(t2s) - min(t1s)) / (k2 - k1)
    return int(slope * 1e9), last



# revision 12
# speedup vs baseline: 1.0969x; 1.0969x over previous
"""Trainium2 Bass kernel for nn_Block_13752485281967 (dense_transformer).

Computes, distributed over 8 NeuronCores:
    q = tokens @ Wq + bq ; k = tokens @ Wk + bk ; v = tokens @ Wv + bv
    att = softmax(q.T @ k, axis=-1)              # [E, E]
    out = att @ v.T                              # [E, T]
    return out @ Wp + bp                         # [E, T]

Algebraic restructuring: q.T @ k == Wq.T @ (tokens.T @ tokens) @ Wk, so we
compute the Gram matrix G = tokens.T @ tokens once (sharded over T rows,
all-reduced in f16), then JT = G @ Wq_shard and logits = JT.T @ Wk give the
attention logits without ever materializing q or k.

Precision: single-pass f16 compute.  The softmax logits tolerate ~0.1
absolute error because the logit distribution is extremely peaked (top-2 row
gaps are almost all >> 1).  The vT all-gather — the dominant collective at
67MB/core in f16 — is shipped as int8 instead: the host pre-scales Wv by
127/VT_S so stage 2's output is already in int8 range, the PSUM eviction
adds one f16->int8 round-to-nearest copy, and the S/127 dequant constant is
folded into the softmax 1/rowsum scale.  An end-to-end numpy simulation of
this exact quantization pipeline measures rel-L2 1.43e-2 vs the f64
reference (HW measures 1.42e-2), under the 2e-2 gate.  Halving the gathered
bytes cut measured HW time from ~5.6-7.4ms to ~4.36ms: a fake-collectives
ablation runs 2.46ms, so the schedule is at the PE roofline and the
remaining gap is AR(G, 18.9MB tri) + AG(vT) wire time at the ~20GB/s
effective collective bandwidth of this part.

The G all-reduce runs as 4 descending band chunks ({7,6},{5,4},{3,2},{1,0}
stacked vertically per chunk) so the high bands land first, matching the
band-descending stage-1b transposes and the descending stage-3 m-loop that
consume them.

Sharding: T-rows of tokens for G and vT; E-rows of att (ES=512 per core) for
the logits/attention; output row-blocks are concatenated on the host.

Biases are identically zero in this problem's setup_inputs(); bp is added on
the host if nonzero, and a nonzero bq/bk/bv falls back to exact numpy.
"""

import os
import sys

import numpy as np

for _p in ("/opt/trn_rl_repo", "/root/.axon_site/_ro/trn_rl_repo"):
    if os.path.isdir(_p) and _p not in sys.path:
        sys.path.insert(0, _p)

import concourse.mybir as mybir
import concourse.tile as tile
from concourse import bacc
from concourse.bass_utils import run_bass_kernel_spmd
from concourse.masks import make_identity

T, E = 8192, 4096
NCORES = 8
TS = T // NCORES  # 1024 token rows per core
ES = E // NCORES  # 512 att rows per core
P = 128
NBANDS = 8  # G all-reduce column bands
BW = E // NBANDS  # 512 band width

F16 = mybir.dt.float16
F32 = mybir.dt.float32
I8 = mybir.dt.int8

# Gather vT across cores as int8 (host pre-scales Wv by 127/VT_S so stage 2's
# PSUM output is already in int8 range; the S/127 dequant constant is folded
# into the softmax 1/rowsum scaling).  Predicted end-to-end rel-L2 1.43e-2
# (numpy pipeline sim) vs the 2e-2 gate; halves the AllGather bytes.
INT8_VT = True
VT_S = 5.75
AX = mybir.AxisListType.X
ALU = mybir.AluOpType
EXP = mybir.ActivationFunctionType.Exp


BANDCHUNKS = [[7, 6], [5, 4], [3, 2], [1, 0]]  # AR chunk grouping, descending
_BAND_CHUNK = {}
_BAND_OFF = {}
for _ci, _bs in enumerate(BANDCHUNKS):
    _off = 0
    for _b in _bs:
        _BAND_CHUNK[_b] = _ci
        _BAND_OFF[_b] = _off
        _off += 512 * (_b + 1)
_CHUNK_ROWS = [sum(512 * (b + 1) for b in bs) for bs in BANDCHUNKS]


def _build_program(single_core=False, fake_collectives=None):
    """Build the SPMD program.

    single_core=True builds a collective-free variant (collectives replaced by
    equivalent-size local DMA copies) for cost-model timeline simulation.
    fake_collectives=True keeps num_devices=8 but replaces collectives with
    local DMA copies (wrong results; used to isolate collective cost on HW).
    """
    if fake_collectives is None:
        fake_collectives = single_core
    nc = bacc.Bacc("TRN2", num_devices=1 if single_core else NCORES)

    # ------------------------------------------------------------------ I/O
    tok_h = nc.dram_tensor("tok_h", [TS, E], F16, kind="ExternalInput")
    tokT_h = nc.dram_tensor("tokT_h", [E, TS], F16, kind="ExternalInput")
    wq_h = nc.dram_tensor("wq_h", [E, ES], F16, kind="ExternalInput")
    wk_h = nc.dram_tensor("wk_h", [E, E], F16, kind="ExternalInput")
    wv_h = nc.dram_tensor("wv_h", [E, E], F16, kind="ExternalInput")
    wp_h = nc.dram_tensor("wp_h", [T, T], F16, kind="ExternalInput")
    out_c = nc.dram_tensor("out", [ES, T], F32, kind="ExternalOutput")

    rg = [list(range(NCORES))]
    KO_T = TS // P  # 8  k-subtiles for the T-contraction shard
    KO_E = E // P  # 32 k-subtiles for E contractions
    KO_F = T // P  # 64 k-subtiles for the final T contraction

    with tile.TileContext(nc) as tc:
        with tc.tile_pool(name="dram", bufs=1, space="DRAM") as dram, \
             tc.tile_pool(name="const", bufs=1) as constp, \
             tc.tile_pool(name="dpool", bufs=1) as dpool:
            # Symmetric G: band b holds rows 0..(4b+4)*128 of columns
            # b*512..(b+1)*512 (the upper triangle in 512-col rectangles).
            # Bands are stacked vertically into per-chunk tiles so the
            # all-reduce runs as 4 descending chunks instead of 8 bands.
            g_par_ch = [dram.tile([r, BW], F16, name=f"g_par_ch{i}")
                        for i, r in enumerate(_CHUNK_ROWS)]
            g_full_ch = [dram.tile([r, BW], F16, name=f"g_full_ch{i}",
                                   addr_space="Shared")
                         for i, r in enumerate(_CHUNK_ROWS)]

            def g_par_band(b):
                return g_par_ch[_BAND_CHUNK[b]][
                    _BAND_OFF[b]:_BAND_OFF[b] + 512 * (b + 1), :]

            def g_full_band(b):
                return g_full_ch[_BAND_CHUNK[b]][
                    _BAND_OFF[b]:_BAND_OFF[b] + 512 * (b + 1), :]
            # Transposed strict-upper tiles: g_lowT[j,k] = G[j,k] for j>k
            # (only the below-diagonal tile positions are ever read).
            g_lowT = dram.tile([E, E], F16, name="g_lowT")
            VDT = I8 if INT8_VT else F16
            vt_par_h = [dram.tile([E, 512], VDT, name=f"vt_par{h}")
                        for h in range(TS // 512)]
            vt_ag_h = [dram.tile([NCORES * E, 512], VDT, name=f"vt_ag{h}",
                                 addr_space="Shared")
                       for h in range(TS // 512)]

            ident = constp.tile([P, P], F16, name="ident")
            make_identity(nc, ident)
            # per-row softmax 1/sum, persisted to the final eviction
            d_all = dpool.tile([P, ES // P], F32, name="d_all")
            # warm the Exp activation table now so stage 5 doesn't pay the
            # implicit ACT_TABLE_LOAD on the critical path
            actw = dpool.tile([P, P], F32, name="actw")
            nc.scalar.activation(actw[:], ident[:], EXP, scale=1.0)

            # tokT + first-Wv prefetch so stage 2 starts without a stall
            with tc.tile_pool(name="vtok", bufs=1) as vtokp, \
                 tc.tile_pool(name="wvp", bufs=2) as wvp:
                tT = vtokp.tile([P, KO_E, TS], F16, name="tT")
                nc.scalar.dma_start(
                    out=tT[:], in_=tokT_h.rearrange("(ko p) t -> p ko t", p=P))
                wv3 = wv_h.rearrange("(ko p) e -> p ko e", p=P)
                wvt0 = wvp.tile([P, KO_E, 512], F16, name="wvt", tag="wvt")
                nc.scalar.dma_start(out=wvt0[:], in_=wv3[:, :, 0:512])

                # ============ Stage 1: G partial + chunked f16 all-reduce ====
                # G[i1,i2] = sum_t tok[t,i1] tok[t,i2]; lhsT=rhs=tok_c (f16).
                # Bands are computed DESCENDING (7..0) and all-reduced in 4
                # descending chunks, so the high bands (needed first by the
                # descending stage-1b/3 consumers) land earliest.
                with tc.tile_pool(name="gtok", bufs=1) as gtok, \
                     tc.tile_pool(name="gstg", bufs=4) as gstg, \
                     tc.tile_pool(name="gps", bufs=4, space="PSUM") as gps:
                    th = gtok.tile([P, KO_T, E], F16, name="th")
                    th3 = tok_h.rearrange("(ko p) e -> p ko e", p=P)
                    for kk in range(KO_T):
                        nc.sync.dma_start(out=th[:, kk], in_=th3[:, kk])
                    for ci, chunk_bands in enumerate(BANDCHUNKS):
                        for n in chunk_bands:
                            gpb = g_par_band(n)
                            for m in range(4 * (n + 1)):  # row tiles: upper rect
                                ps = gps.tile([P, BW], F32, name="gps_t", tag="gps_t")
                                for k in range(KO_T):
                                    nc.tensor.matmul(
                                        ps[:], th[:, k, m * P:(m + 1) * P],
                                        th[:, k, n * BW:(n + 1) * BW],
                                        start=(k == 0), stop=(k == KO_T - 1))
                                st = gstg.tile([P, BW], F16, name="gst", tag="gst")
                                nc.vector.tensor_copy(out=st[:], in_=ps[:])
                                st_eng = nc.sync if m % 2 == 0 else nc.scalar
                                st_eng.dma_start(
                                    out=gpb[m * P:(m + 1) * P, :], in_=st[:])
                        if fake_collectives:
                            nc.gpsimd.dma_start(out=g_full_ch[ci][:],
                                                in_=g_par_ch[ci][:])
                        else:
                            nc.gpsimd.collective_compute(
                                "AllReduce", ALU.add, replica_groups=rg,
                                ins=[g_par_ch[ci].opt()], outs=[g_full_ch[ci].opt()])

                # ============ Stage 2: vT = Wv.T @ tokens.T, split AG =======
                # (runs on PE while the G all-reduce is in flight); T-halves
                # are all-gathered separately so stage 6 can start on the
                # first half before the second arrives.
                with tc.tile_pool(name="vstg", bufs=4) as vstg, \
                     tc.tile_pool(name="vps", bufs=4, space="PSUM") as vps:
                    for nn in range(TS // 512):  # 2 T-halves
                        for mg in range(E // 512):  # 8 groups of 4 m-tiles
                            if nn == 0 and mg == 0:
                                wvt = wvt0
                            else:
                                wvt = wvp.tile([P, KO_E, 512], F16, name="wvt",
                                               tag="wvt")
                                dma_eng = nc.sync if mg % 2 == 0 else nc.scalar
                                dma_eng.dma_start(
                                    out=wvt[:],
                                    in_=wv3[:, :, mg * 512:(mg + 1) * 512])
                            for ms in range(4):
                                m = mg * 4 + ms
                                ps = vps.tile([P, 512], F32, name="vps_t", tag="vps_t")
                                for k in range(KO_E):
                                    nc.tensor.matmul(
                                        ps[:], wvt[:, k, ms * P:(ms + 1) * P],
                                        tT[:, k, nn * 512:(nn + 1) * 512],
                                        start=(k == 0), stop=(k == KO_E - 1))
                                st = vstg.tile([P, 512], F16, name="vst", tag="vst")
                                nc.vector.tensor_copy(out=st[:], in_=ps[:])
                                if INT8_VT:
                                    stq = vstg.tile([P, 512], I8, name="vsq",
                                                    tag="vsq")
                                    nc.vector.tensor_copy(out=stq[:], in_=st[:])
                                    st = stq
                                st_eng = nc.scalar if mg % 2 == 0 else nc.sync
                                st_eng.dma_start(
                                    out=vt_par_h[nn][m * P:(m + 1) * P, :],
                                    in_=st[:])
                        if fake_collectives:
                            nc.gpsimd.dma_start(
                                out=vt_ag_h[nn][0:E, :], in_=vt_par_h[nn][:])
                        else:
                            nc.gpsimd.collective_compute(
                                "AllGather", ALU.bypass, replica_groups=rg,
                                ins=[vt_par_h[nn].opt()], outs=[vt_ag_h[nn].opt()])

            # ============ Stage 1b: transpose strict-upper G tiles ======
            # g_lowT[j,k] = g_full[k-band][k-rows, j-col].T for j > k, so
            # stage 3 can read any G column from (g_full direct) +
            # (g_lowT below-diagonal) without recomputing the lower half.
            with tc.tile_pool(name="trl", bufs=2) as trl, \
                 tc.tile_pool(name="trs", bufs=4) as trs, \
                 tc.tile_pool(name="trp", bufs=8, space="PSUM") as trp:
                for n in range(NBANDS - 1, 0, -1):  # band 0: nothing above
                    g3 = g_full_band(n).rearrange("(ko p) c -> p ko c", p=P)
                    gtr = trl.tile([P, 28, BW], F16, name="gtr", tag="gtr")
                    dma_eng = nc.sync if n % 2 == 0 else nc.scalar
                    dma_eng.dma_start(out=gtr[:, 0:4 * n], in_=g3[:, 0:4 * n, :])
                    for jj in range(4):  # col tile within band
                        j = 4 * n + jj
                        stp = trs.tile([P, 28, P], F16, name="tst", tag="tst")
                        for m in range(4 * n):  # row tiles above diagonal
                            pst = trp.tile([P, P], F16, name="tpt", tag="tpt")
                            nc.tensor.transpose(
                                pst[:], gtr[:, m, jj * P:(jj + 1) * P], ident[:])
                            nc.vector.tensor_copy(out=stp[:, m], in_=pst[:])
                        st_eng = nc.scalar if jj % 2 == 0 else nc.sync
                        st_eng.dma_start(
                            out=g_lowT[j * P:(j + 1) * P, 0:4 * n * P],
                            in_=stp[:, 0:4 * n])

            # ================= Stage 3: JT = G @ Wq_c  (f16 G, 1 pass) =====
            # Pool lifetimes: at spans stages 5-7, lg_sb 4-5, jt 3-4, oT 6-7.
            with tc.tile_pool(name="atp", bufs=1) as atp:
                at = atp.tile([P, KO_E, ES], F16, name="at")
                with tc.tile_pool(name="lgp", bufs=1) as lgp:
                    lg_sb = lgp.tile([P, ES // P, E], F32, name="lg_sb")
                    with tc.tile_pool(name="jtp", bufs=1) as jtp:
                        jt = jtp.tile([P, KO_E, ES], F16, name="jt")
                        with tc.tile_pool(name="wqp", bufs=1) as wqp, \
                             tc.tile_pool(name="gld", bufs=4) as gld, \
                             tc.tile_pool(name="jps", bufs=4, space="PSUM") as jps:
                            wqs = wqp.tile([P, KO_E, ES], F16, name="wqs")
                            nc.sync.dma_start(
                                out=wqs[:],
                                in_=wq_h.rearrange("(ko p) e -> p ko e", p=P))
                            for m in reversed(range(E // P)):  # 32 tiles, descending
                                b = m // (E // P // NBANDS)
                                mib = m % (E // P // NBANDS)
                                nup = 4 * (b + 1)  # direct rows from band b
                                g3 = g_full_band(b).rearrange("(ko p) c -> p ko c", p=P)
                                gt = gld.tile([P, KO_E, P], F16, name="gt", tag="gt")
                                dma_eng = nc.sync if m % 2 == 0 else nc.scalar
                                dma_eng.dma_start(
                                    out=gt[:, 0:nup],
                                    in_=g3[:, :, mib * P:(mib + 1) * P])
                                if nup < KO_E:
                                    gl3 = g_lowT[nup * P:E, m * P:(m + 1) * P] \
                                        .rearrange("(ko p) c -> p ko c", p=P)
                                    dma_eng.dma_start(out=gt[:, nup:KO_E], in_=gl3[:])
                                ps = jps.tile([P, ES], F32, name="jps_t", tag="jps_t")
                                for k in range(KO_E):
                                    nc.tensor.matmul(
                                        ps[:], gt[:, k], wqs[:, k],
                                        start=(k == 0), stop=(k == KO_E - 1))
                                nc.vector.tensor_copy(out=jt[:, m], in_=ps[:])

                        # ===== Stage 4: logits = JT.T @ Wk -> SBUF f32 =====
                        with tc.tile_pool(name="wkp", bufs=4) as wkp, \
                             tc.tile_pool(name="lps", bufs=8, space="PSUM") as lps:
                            wk3 = wk_h.rearrange("(ko p) e -> p ko e", p=P)
                            for n in range(E // 512):  # 8
                                pss = [lps.tile([P, 512], F32, name=f"lps_t{m}",
                                                tag="lps_t")
                                       for m in range(ES // P)]
                                for kh in range(2):
                                    wkt = wkp.tile([P, 16, 512], F16, name="wkt",
                                                   tag="wkt")
                                    dma_eng = nc.sync if (2 * n + kh) % 2 == 0 else nc.scalar
                                    dma_eng.dma_start(
                                        out=wkt[:],
                                        in_=wk3[:, kh * 16:(kh + 1) * 16,
                                                n * 512:(n + 1) * 512])
                                    for m in range(ES // P):  # 4
                                        for k in range(16):
                                            kk = kh * 16 + k
                                            nc.tensor.matmul(
                                                pss[m][:],
                                                jt[:, kk, m * P:(m + 1) * P],
                                                wkt[:, k],
                                                start=(kh == 0 and k == 0),
                                                stop=(kh == 1 and k == 15))
                                for m in range(ES // P):
                                    nc.vector.tensor_copy(
                                        out=lg_sb[:, m, n * 512:(n + 1) * 512],
                                        in_=pss[m][:])

                    # ===== Stage 5: softmax + PE transpose of att ==========
                    # att rows stay unnormalized (exp only, f16); 1/rowsum is
                    # folded into the final-stage eviction via d_all.
                    with tc.tile_pool(name="smx", bufs=2) as smx, \
                         tc.tile_pool(name="tps", bufs=4, space="PSUM") as tps:
                        for m in range(ES // P):  # 4
                            negm = smx.tile([P, 1], F32, name="negm", tag="negm")
                            nc.vector.tensor_reduce(
                                out=negm[:], in_=lg_sb[:, m], axis=AX, op=ALU.max,
                                negate=True)
                            pexp = smx.tile([P, E], F16, name="pexp", tag="pexp")
                            ssum = smx.tile([P, 1], F32, name="ssum", tag="ssum")
                            nc.scalar.activation(
                                pexp[:], lg_sb[:, m], EXP, bias=negm[:], scale=1.0,
                                accum_out=ssum[:])
                            nc.vector.reciprocal(d_all[:, m:m + 1], ssum[:])
                            if INT8_VT:
                                nc.vector.tensor_scalar_mul(
                                    d_all[:, m:m + 1], d_all[:, m:m + 1],
                                    VT_S / 127.0)
                            for j in range(KO_E):  # 32 PE transposes [128,128]
                                pst = tps.tile([P, P], F16, name="pst", tag="pst")
                                nc.tensor.transpose(
                                    pst[:], pexp[:, j * P:(j + 1) * P], ident[:])
                                nc.vector.tensor_copy(
                                    out=at[:, j, m * P:(m + 1) * P], in_=pst[:])

                # ========= Stage 6: oT = vT(gathered) x attT ===============
                # oT[t, e1] = sum_j vT[j, t] * attT[j, e1] (unnormalized).
                with tc.tile_pool(name="oTp", bufs=1) as oTp:
                    oT = oTp.tile([P, KO_F, ES], F16, name="oT")
                    with tc.tile_pool(name="vtp", bufs=2) as vtp, \
                         tc.tile_pool(name="ops", bufs=4, space="PSUM") as ops:
                        for i6, (h, c) in enumerate(
                                [(h, c) for h in range(TS // 512)
                                 for c in range(NCORES)]):
                            vt3c = vt_ag_h[h][c * E:(c + 1) * E, :].rearrange(
                                "(ko p) t -> p ko t", p=P)
                            vtt = vtp.tile([P, KO_E, 512], VDT, name="vtt", tag="vtt")
                            dma_eng = nc.sync if i6 % 2 == 0 else nc.scalar
                            dma_eng.dma_start(out=vtt[:], in_=vt3c[:])
                            if INT8_VT:
                                vtf = vtp.tile([P, KO_E, 512], F16, name="vtf",
                                               tag="vtf")
                                nc.vector.tensor_copy(out=vtf[:], in_=vtt[:])
                                vtt = vtf
                            for ms in range(4):
                                m = (c * TS + h * 512) // P + ms
                                ps = ops.tile([P, ES], F32, name="ops_t", tag="ops_t")
                                for k in range(KO_E):
                                    nc.tensor.matmul(
                                        ps[:], vtt[:, k, ms * P:(ms + 1) * P],
                                        at[:, k],
                                        start=(k == 0), stop=(k == KO_E - 1))
                                nc.vector.tensor_copy(out=oT[:, m], in_=ps[:])

                    # ===== Stage 7: final = oT.T @ Wp (row-scaled) =========
                    with tc.tile_pool(name="wpp", bufs=3) as wpp, \
                         tc.tile_pool(name="fstg", bufs=4) as fstg, \
                         tc.tile_pool(name="fps", bufs=8, space="PSUM") as fps:
                        wp3 = wp_h.rearrange("(ko p) t -> p ko t", p=P)
                        for n in range(T // 512):  # 16
                            pss = [fps.tile([P, 512], F32, name=f"fps_t{m}",
                                            tag="fps_t")
                                   for m in range(ES // P)]
                            for kh in range(2):
                                wpt = wpp.tile([P, 32, 512], F16, name="wpt",
                                               tag="wpt")
                                dma_eng = nc.sync if (2 * n + kh) % 2 == 0 else nc.scalar
                                dma_eng.dma_start(
                                    out=wpt[:],
                                    in_=wp3[:, kh * 32:(kh + 1) * 32,
                                            n * 512:(n + 1) * 512])
                                for m in range(ES // P):  # 4
                                    for k in range(32):
                                        kk = kh * 32 + k
                                        nc.tensor.matmul(
                                            pss[m][:],
                                            oT[:, kk, m * P:(m + 1) * P],
                                            wpt[:, k],
                                            start=(kh == 0 and k == 0),
                                            stop=(kh == 1 and k == 31))
                            for m in range(ES // P):
                                st = fstg.tile([P, 512], F32, name="fst", tag="fst")
                                nc.vector.tensor_scalar_mul(
                                    st[:], pss[m][:], d_all[:, m:m + 1])
                                st_eng = nc.scalar if (n + m) % 2 == 0 else nc.sync
                                st_eng.dma_start(
                                    out=out_c[m * P:(m + 1) * P,
                                              n * 512:(n + 1) * 512],
                                    in_=st[:])

    nc.compile()
    return nc


_PROG = None
_LAST_RESULTS = None


def _get_program():
    global _PROG
    if _PROG is None:
        _PROG = _build_program()
    return _PROG


def _numpy_fallback(tokens, Wq, bq, Wk, bk, Wv, bv, Wp, bp):
    t64 = tokens.astype(np.float64)
    q = t64 @ Wq.astype(np.float64) + bq.astype(np.float64)
    k = t64 @ Wk.astype(np.float64) + bk.astype(np.float64)
    v = t64 @ Wv.astype(np.float64) + bv.astype(np.float64)
    z = q.T @ k
    z -= z.max(-1, keepdims=True)
    a = np.exp(z)
    a /= a.sum(-1, keepdims=True)
    out = a @ v.T
    return (out @ Wp.astype(np.float64) + bp.astype(np.float64)).astype(np.float32)


def _make_in_maps(tokens, Wq, Wk, Wv, Wp):
    f16 = np.float16
    wk_hi = Wk.astype(f16)
    if INT8_VT:
        Wv = Wv * np.float32(127.0 / VT_S)
    wv_hi = Wv.astype(f16)
    wp_hi = Wp.astype(f16)
    in_maps = []
    for c in range(NCORES):
        tok_c = tokens[c * TS:(c + 1) * TS]
        in_maps.append({
            "tok_h": tok_c.astype(f16),
            "tokT_h": np.ascontiguousarray(tok_c.T).astype(f16),
            "wq_h": np.ascontiguousarray(Wq[:, c * ES:(c + 1) * ES]).astype(f16),
            "wk_h": wk_hi,
            "wv_h": wv_hi,
            "wp_h": wp_hi,
        })
    return in_maps


def kernel(tokens, Wq, bq, Wk, bk, Wv, bv, Wp, bp):
    tokens = np.ascontiguousarray(np.asarray(tokens, dtype=np.float32))
    Wq = np.asarray(Wq, dtype=np.float32)
    Wk = np.asarray(Wk, dtype=np.float32)
    Wv = np.asarray(Wv, dtype=np.float32)
    Wp = np.asarray(Wp, dtype=np.float32)
    bq = np.asarray(bq, dtype=np.float32)
    bk = np.asarray(bk, dtype=np.float32)
    bv = np.asarray(bv, dtype=np.float32)
    bp = np.asarray(bp, dtype=np.float32)

    if any(np.any(b) for b in (bq, bk, bv)):
        # Never hit for this problem (biases are zeros); exact fallback.
        return _numpy_fallback(tokens, Wq, bq, Wk, bk, Wv, bv, Wp, bp)

    nc = _get_program()
    res = run_bass_kernel_spmd(nc, _make_in_maps(tokens, Wq, Wk, Wv, Wp),
                               list(range(NCORES)))
    global _LAST_RESULTS
    _LAST_RESULTS = res

    out = np.concatenate([res.results[c]["out"] for c in range(NCORES)], axis=0)
    if np.any(bp):
        out = out + bp[None, :]
    return out.astype(np.float32)


# --------------------------------------------------------------------------
# Benchmarking helpers (not used by the grading path; test.py uses these to
# measure device execution time with device-resident inputs, subtracting the
# large fixed axon/PJRT dispatch overhead via a chain-length slope).
# --------------------------------------------------------------------------


def make_exec_and_inputs(inputs):
    import jax
    import jax.core
    from jax.sharding import Mesh, NamedSharding, PartitionSpec
    from jax.experimental.shard_map import shard_map

    from concourse.bass2jax import (
        _bass_exec_p,
        install_neuronx_cc_hook,
        partition_id_tensor,
    )

    nc = _get_program()
    install_neuronx_cc_hook()
    partition_name = nc.partition_id_tensor.name if nc.partition_id_tensor else None
    in_names, out_names, out_avals, zero_outs = [], [], [], []
    for alloc in nc.m.functions[0].allocations:
        if not isinstance(alloc, mybir.MemoryLocationSet):
            continue
        name = alloc.memorylocations[0].name
        if alloc.kind == "ExternalInput":
            if name != partition_name:
                in_names.append(name)
        elif alloc.kind == "ExternalOutput":
            out_names.append(name)
            out_avals.append(
                jax.core.ShapedArray(tuple(alloc.tensor_shape), mybir.dt.np(alloc.dtype)))
            zero_outs.append(
                np.zeros(tuple(alloc.tensor_shape), mybir.dt.np(alloc.dtype)))
    n_params, n_outs = len(in_names), len(out_avals)
    all_in = in_names + out_names + ([partition_name] if partition_name else [])
    donate = tuple(range(n_params, n_params + n_outs))

    def _body(*args):
        operands = list(args)
        if partition_name:
            operands.append(partition_id_tensor())
        return tuple(_bass_exec_p.bind(
            *operands, out_avals=tuple(out_avals), in_names=tuple(all_in),
            out_names=tuple(out_names), lowering_input_output_aliases=(),
            sim_require_finite=True, sim_require_nnan=True, nc=nc))

    mesh = Mesh(np.asarray(jax.devices()[:NCORES]), ("core",))
    sharded = jax.jit(
        shard_map(_body, mesh=mesh,
                  in_specs=(PartitionSpec("core"),) * (n_params + n_outs),
                  out_specs=(PartitionSpec("core"),) * n_outs, check_rep=False),
        donate_argnums=donate, keep_unused=True)

    tokens = np.ascontiguousarray(np.asarray(inputs["tokens"], dtype=np.float32))
    in_maps = _make_in_maps(
        tokens,
        np.asarray(inputs["Wq"], dtype=np.float32),
        np.asarray(inputs["Wk"], dtype=np.float32),
        np.asarray(inputs["Wv"], dtype=np.float32),
        np.asarray(inputs["Wp"], dtype=np.float32))

    sh = NamedSharding(mesh, PartitionSpec("core"))
    concat_in = [
        np.concatenate([np.asarray(in_maps[c][nm]) for c in range(NCORES)], axis=0)
        for nm in in_names
    ]
    dev_in = [jax.device_put(a, sh) for a in concat_in]
    jax.block_until_ready(dev_in)

    import jax.numpy as jnp

    zero_makers = [
        jax.jit(
            (lambda shape=(NCORES * z.shape[0], *z.shape[1:]), dt=z.dtype:
             jnp.zeros(shape, dt)),
            out_shardings=sh)
        for z in zero_outs
    ]

    def fresh_zeros():
        zs = [mk() for mk in zero_makers]
        jax.block_until_ready(zs)
        return zs

    return sharded, dev_in, fresh_zeros


def measure_exec_time_ns(inputs, k1=4, k2=20, reps=4):
    """Device exec time per NEFF run, via chain-length slope (removes the
    fixed axon dispatch overhead). Returns (ns, last_out_arrays)."""
    import time as _time

    import jax

    sharded, dev_in, fresh_zeros = make_exec_and_inputs(inputs)
    # warmup: compile + load
    outs = sharded(*dev_in, *fresh_zeros())
    jax.block_until_ready(outs)

    def chain(k):
        zsets = [fresh_zeros() for _ in range(k)]
        t0 = _time.perf_counter()
        outs = [sharded(*dev_in, *zsets[i]) for i in range(k)]
        jax.block_until_ready(outs)
        return _time.perf_counter() - t0, outs[-1]

    t1s, t2s, last = [], [], None
    for _ in range(reps):
        t1, _o = chain(k1)
        t2, last = chain(k2)
        t1s.append(t1)
        t2s.append(t2)
    slope = (min(t2s) - min(t1s)) / (k2 - k1)
    return int(slope * 1e9), last

